# revision 17
# baseline (speedup 1.0000x reference)
"""Multi-head attention with KV cache, sharded over 8 NeuronCores by head.

Problem (hardcoded shapes):
  x       [4, 512, 1024]      hidden states (B, T, D)
  k_prev  [4, 16, 3584, 64]   KV cache (B, H, PAST, HD)
  v_prev  [4, 16, 3584, 64]
  Wq/Wk/Wv/Wo [1024, 1024]    projection weights (torch Linear: y = x @ W.T)

Sharding: 16 heads / 8 cores = 2 heads per core (data stays full along batch).
Each core computes q/k/v projections for its 2 heads (column-parallel),
full attention for its heads, and a column-parallel o_proj partial
[2048, 1024]; the host sums the 8 partials (the o_proj all-reduce).

Device algorithm per core (all fp32):
  - qT/kT_new/vT_new = W_slice @ x^T   via PE, contracting D (xT fed from host)
  - per (batch, head): scores^T[key, q] = k @ q^T (K=HD=64 on partitions),
    streamed in key-chunks of 128; causal mask added on the 4 newest chunks;
    exp on ScalarE (no max subtraction -- scores are O(1) by construction);
    out^T[hd, q] accumulated in PSUM via lhsT = [v | 1] so row 64 of the
    accumulator is the softmax denominator; divide, then o_proj.
"""

import numpy as np

import concourse.bass as bass
import concourse.mybir as mybir
import concourse.tile as tile
from concourse import bacc
from concourse.bass_utils import run_bass_kernel_spmd
from concourse.masks import make_identity

B, T, D = 4, 512, 1024
H, HD = 16, 64
PAST = 3584
L = PAST + T            # 4096 == MAX_CACHE, nothing is trimmed
SCALE = 1.0 / np.sqrt(HD).astype(np.float32)
NCORES = 8
HPC = H // NCORES       # heads per core = 2
TOK = B * T             # 2048
NCH = L // 128          # 32 key chunks per (b, h)
PCH = PAST // 128       # 28 chunks from the cache
FP32 = mybir.dt.float32
NEG = -1.0e30

_cache = {}

# float32r: 4-byte fp32 variant the PE consumes at full rate (~12-13 mantissa
# bits effective, measured) vs 4 cycles/row for fp32. All matmul operands are
# stored as fp32r; producers (DMA / DVE copy / ACT exp) write the rounded form.
FP32R = mybir.dt.float32r


def _build():
    nc = bacc.Bacc(None, target_bir_lowering=False)

    xT = nc.dram_tensor("xT", [D, TOK], FP32R, kind="ExternalInput")
    wqT = nc.dram_tensor("wqT", [D, 128], FP32R, kind="ExternalInput")
    wkT = nc.dram_tensor("wkT", [D, 128], FP32R, kind="ExternalInput")
    wvT = nc.dram_tensor("wvT", [D, 128], FP32R, kind="ExternalInput")
    woT = nc.dram_tensor("woT", [128, D], FP32R, kind="ExternalInput")
    kTp = nc.dram_tensor("kTp", [B, 128, PAST], FP32R, kind="ExternalInput")
    vp = nc.dram_tensor("vp", [B, 128, HPC, PCH, HD + 1], FP32R, kind="ExternalInput")
    out = nc.dram_tensor("out", [TOK, D], FP32, kind="ExternalOutput")

    Exp = mybir.ActivationFunctionType.Exp
    mult = mybir.AluOpType.mult
    add = mybir.AluOpType.add

    # key-chunk groups: scores psum tiles hold up to 3 chunks (3 PSUM banks)
    groups = [list(range(s, min(s + 3, NCH))) for s in range(0, NCH, 3)]

    with tile.TileContext(nc) as tc:
        with (
            tc.tile_pool(name="const", bufs=1) as const,
            tc.tile_pool(name="persist", bufs=1) as persist,
            tc.tile_pool(name="kv", bufs=2) as kv,
            tc.tile_pool(name="pt", bufs=3) as ptp,
            tc.tile_pool(name="div", bufs=2) as divp,
            tc.tile_pool(name="stage", bufs=1) as stage,
            tc.tile_pool(name="acc_ps", bufs=1, space="PSUM") as accp,
            tc.tile_pool(name="flex_ps", bufs=1, space="PSUM") as flexp,
        ):
            # ---- constants ----
            identity = const.tile([128, 128], FP32)
            make_identity(nc, identity)
            identity_r = const.tile([128, 128], FP32R)
            nc.vector.tensor_copy(identity_r, identity)
            masks = []
            for r in range(4):
                m = const.tile([128, T], FP32, tag=f"mask{r}")
                nc.gpsimd.memset(m, 0.0)
                # keep 0 where query i >= key-token (128r + kk), else NEG
                nc.gpsimd.affine_select(
                    out=m, in_=m, compare_op=mybir.AluOpType.is_ge,
                    fill=NEG, base=-128 * r, channel_multiplier=-1,
                    pattern=[[1, T]],
                )
                mr = const.tile([128, T], FP32R, tag=f"maskr{r}", name=f"maskr{r}")
                nc.vector.tensor_copy(mr, m)
                masks.append(mr)

            ones_c = const.tile([128, 1], FP32)
            nc.gpsimd.memset(ones_c, 1.0)
            warm = const.tile([1, 1], FP32)
            nc.scalar.activation(warm, ones_c[:1, :], Exp)

            # ---- persistent SBUF ----
            woT_s = persist.tile([128, D], FP32R)
            qT = persist.tile([128, TOK], FP32R, tag="qT")
            kTn = persist.tile([128, TOK], FP32, tag="kTn")
            vTn = persist.tile([128, TOK], FP32, tag="vTn")
            oT = persist.tile([128, TOK], FP32R, tag="oT")

            def setup_batch(b, kT=None, va=None):
                bsl = bass.ts(b, T)
                if kT is None:
                    kT = kv.tile([128, L], FP32R, tag="kT", name=f"kT{b}")
                    nc.sync.dma_start(kT[:, :PAST], kTp[b, :, :])
                nc.vector.tensor_copy(kT[:, PAST:], kTn[:, bsl])
                if va is None:
                    va = kv.tile(
                        [128, HPC, NCH, HD + 1], FP32R, tag="va", name=f"va{b}"
                    )
                    nc.sync.dma_start(va[:, :, :PCH, :], vp[b, :, :, :, :])
                nc.vector.tensor_copy(
                    va[:, :, PCH:, HD],
                    ones_c[:, :, None].to_broadcast([128, HPC, NCH - PCH]),
                )
                for h in range(HPC):
                    hsl = slice(h * HD, (h + 1) * HD)
                    for tt in range(T // 128):
                        tp = flexp.tile([128, 512], FP32, tag="flex")
                        nc.tensor.transpose(
                            tp[:, :HD],
                            vTn[hsl, b * T + tt * 128 : b * T + (tt + 1) * 128],
                            identity[hsl, hsl],
                        )
                        nc.vector.tensor_copy(va[:, h, PCH + tt, :HD], tp[:, :HD])
                return kT, va

            # ---- phase A: projections (q/k/v for this core's 2 heads) ----
            nxt = None
            with (
                tc.tile_pool(name="xw", bufs=1) as xw,
                tc.tile_pool(name="xs", bufs=2) as xs,
            ):
                xT_r = xT.rearrange("(ko p) t -> p ko t", p=128)
                w_s = {}
                for name, w in (("q", wqT), ("k", wkT), ("v", wvT)):
                    w_s[name] = xw.tile(
                        [128, D // 128, 128], FP32R, tag=f"w{name}", name=f"w{name}"
                    )
                    if name == "q":
                        nc.sync.dma_start(
                            w_s[name], w.rearrange("(ko p) m -> p ko m", p=128)
                        )
                        xT_s0 = xs.tile([128, D // 128, 512], FP32R, tag="xT")
                        half = D // 256
                        nc.sync.dma_start(
                            xT_s0[:, :half, :], xT_r[:, :half, :512]
                        )
                        nc.sync.dma_start(
                            xT_s0[:, half:, :], xT_r[:, half:, :512]
                        )
                kT0 = kv.tile([128, L], FP32R, tag="kT", name="kT0")
                nc.sync.dma_start(kT0[:, : 12 * 128], kTp[0, :, : 12 * 128])
                va0 = kv.tile(
                    [128, HPC, NCH, HD + 1], FP32R, tag="va", name="va0"
                )
                nc.sync.dma_start(va0[:, :, :12, :], vp[0, :, :, :12, :])
                for name, w in (("k", wkT), ("v", wvT)):
                    nc.sync.dma_start(
                        w_s[name], w.rearrange("(ko p) m -> p ko m", p=128)
                    )
                nc.sync.dma_start(kT0[:, 12 * 128 : PAST], kTp[0, :, 12 * 128 :])
                nc.sync.dma_start(va0[:, :, 12:PCH, :], vp[0, :, :, 12:, :])
                for tcn in range(TOK // 512):
                    if tcn == 0:
                        xT_s = xT_s0
                    else:
                        xT_s = xs.tile([128, D // 128, 512], FP32R, tag="xT")
                        half = D // 256
                        nc.sync.dma_start(
                            xT_s[:, :half, :], xT_r[:, :half, bass.ts(tcn, 512)]
                        )
                        nc.sync.dma_start(
                            xT_s[:, half:, :], xT_r[:, half:, bass.ts(tcn, 512)]
                        )
                    for name, dst in (("q", qT), ("k", kTn), ("v", vTn)):
                        ps = flexp.tile([128, 512], FP32, tag="flex")
                        for ko in range(D // 128):
                            nc.tensor.matmul(
                                ps,
                                lhsT=w_s[name][:, ko, :],
                                rhs=xT_s[:, ko, :],
                                start=(ko == 0),
                                stop=(ko == D // 128 - 1),
                            )
                        nc.vector.tensor_copy(dst[:, bass.ts(tcn, 512)], ps)
                    if tcn == 0:
                        nxt = setup_batch(0, kT=kT0, va=va0)

            nc.sync.dma_start(woT_s, woT[:, :])

            # ---- phase B: attention per (batch, head) ----
            with tc.tile_pool(name="sc_ps", bufs=2, space="PSUM") as scp:
                for b in range(B):
                    bsl = bass.ts(b, T)
                    kT, va = nxt
                    if b + 1 < B:
                        nxt = setup_batch(b + 1)

                    for h in range(HPC):
                        hsl = slice(h * HD, (h + 1) * HD)
                        acc = accp.tile([HD + 1, 512], FP32, tag="acc")
                        qTh = qT[hsl, bsl]
                        for g in groups:
                            ng = len(g)
                            ps = scp.tile([128, 3 * 512], FP32, tag="sc")
                            for j, cc in enumerate(g):
                                masked = cc >= PCH
                                # queries < off see nothing from chunk cc
                                off = max(0, (cc - PCH) * 128)
                                nc.tensor.matmul(
                                    ps[:, j * 512 + off : (j + 1) * 512],
                                    lhsT=kT[hsl, bass.ts(cc, 128)],
                                    rhs=qTh[:, off:],
                                    start=True,
                                    stop=not masked,
                                )
                                if masked:
                                    nc.tensor.matmul(
                                        ps[:, j * 512 + off : (j + 1) * 512],
                                        lhsT=identity_r,
                                        rhs=masks[cc - PCH][:, off:],
                                        start=False,
                                        stop=True,
                                        skip_group_check=True,
                                    )
                            pT = ptp.tile([128, 3 * 512], FP32R, tag="pT")
                            nc.scalar.activation(
                                pT[:, : ng * 512], ps[:, : ng * 512], Exp
                            )
                            for j, cc in enumerate(g):
                                off = max(0, (cc - PCH) * 128)
                                nc.tensor.matmul(
                                    acc[:, off:],
                                    lhsT=va[:, h, cc, :],
                                    rhs=pT[:, j * 512 + off : (j + 1) * 512],
                                    start=(cc == 0),
                                    stop=(cc == NCH - 1),
                                    skip_group_check=True,
                                )
                        # evict accumulator to SBUF at once (frees the PSUM
                        # bank for the next head); denominator in row 64
                        asb = divp.tile([HD + 1, 512], FP32, tag="asb")
                        nc.vector.tensor_copy(asb, acc)
                        r0 = divp.tile([1, 512], FP32, tag="r0")
                        nc.vector.reciprocal(r0, asb[HD : HD + 1, :])
                        bc = divp.tile([HD, 512], FP32, tag="bc")
                        nc.gpsimd.partition_broadcast(bc, r0)
                        nc.vector.tensor_tensor(
                            oT[hsl, bsl], asb[:HD, :], bc, mult
                        )

                    # ---- column-parallel o_proj for this batch ----
                    ost = stage.tile([128, T // 128, D], FP32, tag="ost")
                    out_r = out[bsl, :].rearrange("(tt p) d -> p tt d", p=128)
                    for tt in range(T // 128):
                        tsl = slice(b * T + tt * 128, b * T + (tt + 1) * 128)
                        for nh in range(2):
                            if b == B - 1:
                                ps = scp.tile([128, 3 * 512], FP32, tag="sc")
                                ps = ps[:, :512]
                            else:
                                ps = flexp.tile([128, 512], FP32, tag="flex")
                            nc.tensor.matmul(
                                ps,
                                lhsT=oT[:, tsl],
                                rhs=woT_s[:, bass.ts(nh, 512)],
                                start=True,
                                stop=True,
                            )
                            if b == B - 1 and nh == 1:
                                nc.scalar.copy(ost[:, tt, bass.ts(nh, 512)], ps)
                            else:
                                nc.vector.tensor_copy(
                                    ost[:, tt, bass.ts(nh, 512)], ps
                                )
                        nc.sync.dma_start(out_r[:, tt, :], ost[:, tt, :])

    nc.compile()
    return nc


def _pack_v(v):
    """[B, HPC, PAST, HD] -> [B, 128, HPC, PCH, HD+1] with ones in col HD."""
    out = np.empty((B, 128, HPC, PCH, HD + 1), np.float32)
    # v[b, h, c*128 + p, hd] -> out[b, p, h, c, hd]
    out[..., :HD] = v.reshape(B, HPC, PCH, 128, HD).transpose(0, 3, 1, 2, 4)
    out[..., HD] = 1.0
    return np.ascontiguousarray(out)


def _prep(x, k_prev, v_prev, Wq, Wk, Wv, Wo):
    """Host-side shard + layout marshalling (fp32, C-contiguous)."""
    f = np.float32
    x2 = np.ascontiguousarray(np.asarray(x, f).reshape(TOK, D))
    xT = np.ascontiguousarray(x2.T)
    k_prev = np.asarray(k_prev, f)
    v_prev = np.asarray(v_prev, f)
    Wq, Wk, Wv, Wo = (np.asarray(w, f) for w in (Wq, Wk, Wv, Wo))
    in_maps = []
    for c in range(NCORES):
        rows = slice(128 * c, 128 * (c + 1))
        hsl = slice(HPC * c, HPC * (c + 1))
        in_maps.append(
            {
                "xT": xT,
                "wqT": np.ascontiguousarray((Wq[rows, :] * SCALE).T),
                "wkT": np.ascontiguousarray(Wk[rows, :].T),
                "wvT": np.ascontiguousarray(Wv[rows, :].T),
                "woT": np.ascontiguousarray(Wo[:, rows].T),
                "kTp": np.ascontiguousarray(
                    k_prev[:, hsl, :, :].transpose(0, 1, 3, 2)
                ).reshape(B, 128, PAST),
                "vp": _pack_v(v_prev[:, hsl, :, :]),
            }
        )
    return in_maps


def kernel(x, k_prev, v_prev, Wq, Wk, Wv, Wo):
    if "nc" not in _cache:
        _cache["nc"] = _build()
    nc = _cache["nc"]
    in_maps = _prep(x, k_prev, v_prev, Wq, Wk, Wv, Wo)
    res = run_bass_kernel_spmd(nc, in_maps, core_ids=list(range(NCORES)))
    acc = np.zeros((TOK, D), np.float64)
    for r in res.results:
        acc += r["out"]
    return acc.astype(np.float32).reshape(B, T, D)


# revision 21
# speedup vs baseline: 1.0027x; 1.0027x over previous
"""Multi-head attention with KV cache, sharded over 8 NeuronCores by head.

Problem (hardcoded shapes):
  x       [4, 512, 1024]      hidden states (B, T, D)
  k_prev  [4, 16, 3584, 64]   KV cache (B, H, PAST, HD)
  v_prev  [4, 16, 3584, 64]
  Wq/Wk/Wv/Wo [1024, 1024]    projection weights (torch Linear: y = x @ W.T)

Sharding: 16 heads / 8 cores = 2 heads per core (data stays full along batch).
Each core computes q/k/v projections for its 2 heads (column-parallel),
full attention for its heads, and a column-parallel o_proj partial
[2048, 1024]; the host sums the 8 partials (the o_proj all-reduce).

Device algorithm per core (all fp32):
  - qT/kT_new/vT_new = W_slice @ x^T   via PE, contracting D (xT fed from host)
  - per (batch, head): scores^T[key, q] = k @ q^T (K=HD=64 on partitions),
    streamed in key-chunks of 128; causal mask added on the 4 newest chunks;
    exp on ScalarE (no max subtraction -- scores are O(1) by construction);
    out^T[hd, q] accumulated in PSUM via lhsT = [v | 1] so row 64 of the
    accumulator is the softmax denominator; divide, then o_proj.
"""

import numpy as np

import concourse.bass as bass
import concourse.mybir as mybir
import concourse.tile as tile
from concourse import bacc
from concourse.bass_utils import run_bass_kernel_spmd
from concourse.masks import make_identity

B, T, D = 4, 512, 1024
H, HD = 16, 64
PAST = 3584
L = PAST + T            # 4096 == MAX_CACHE, nothing is trimmed
SCALE = 1.0 / np.sqrt(HD).astype(np.float32)
NCORES = 8
HPC = H // NCORES       # heads per core = 2
TOK = B * T             # 2048
NCH = L // 128          # 32 key chunks per (b, h)
PCH = PAST // 128       # 28 chunks from the cache
FP32 = mybir.dt.float32
NEG = -1.0e30

_cache = {}

# float32r: 4-byte fp32 variant the PE consumes at full rate (~12-13 mantissa
# bits effective, measured) vs 4 cycles/row for fp32. All matmul operands are
# stored as fp32r; producers (DMA / DVE copy / ACT exp) write the rounded form.
FP32R = mybir.dt.float32r


def _build():
    nc = bacc.Bacc(None, target_bir_lowering=False)

    xT = nc.dram_tensor("xT", [D, TOK], FP32R, kind="ExternalInput")
    wqT = nc.dram_tensor("wqT", [D, 128], FP32R, kind="ExternalInput")
    wkT = nc.dram_tensor("wkT", [D, 128], FP32R, kind="ExternalInput")
    wvT = nc.dram_tensor("wvT", [D, 128], FP32R, kind="ExternalInput")
    woT = nc.dram_tensor("woT", [128, D], FP32R, kind="ExternalInput")
    kTp = nc.dram_tensor("kTp", [B, 128, PAST], FP32R, kind="ExternalInput")
    vp = nc.dram_tensor("vp", [B, 128, HPC, PCH, HD + 1], FP32R, kind="ExternalInput")
    out = nc.dram_tensor("out", [TOK, D], FP32, kind="ExternalOutput")

    Exp = mybir.ActivationFunctionType.Exp
    mult = mybir.AluOpType.mult
    add = mybir.AluOpType.add

    # key-chunk groups: scores psum tiles hold up to 3 chunks (3 PSUM banks)
    groups = [list(range(s, min(s + 3, NCH))) for s in range(0, NCH, 3)]

    with tile.TileContext(nc) as tc:
        with (
            tc.tile_pool(name="const", bufs=1) as const,
            tc.tile_pool(name="persist", bufs=1) as persist,
            tc.tile_pool(name="kv", bufs=2) as kv,
            tc.tile_pool(name="pt", bufs=2) as ptp,
            tc.tile_pool(name="div", bufs=2) as divp,
            tc.tile_pool(name="stage", bufs=1) as stage,
            tc.tile_pool(name="acc_ps", bufs=1, space="PSUM") as accp,
            tc.tile_pool(name="flex_ps", bufs=1, space="PSUM") as flexp,
        ):
            # ---- constants ----
            identity = const.tile([128, 128], FP32)
            make_identity(nc, identity)
            identity_r = const.tile([128, 128], FP32R)
            nc.vector.tensor_copy(identity_r, identity)
            masks = []
            for r in range(4):
                m = const.tile([128, T], FP32, tag=f"mask{r}")
                nc.gpsimd.memset(m, 0.0)
                # keep 0 where query i >= key-token (128r + kk), else NEG
                nc.gpsimd.affine_select(
                    out=m, in_=m, compare_op=mybir.AluOpType.is_ge,
                    fill=NEG, base=-128 * r, channel_multiplier=-1,
                    pattern=[[1, T]],
                )
                mr = const.tile([128, T], FP32R, tag=f"maskr{r}", name=f"maskr{r}")
                nc.vector.tensor_copy(mr, m)
                masks.append(mr)

            ones_c = const.tile([128, 1], FP32)
            nc.gpsimd.memset(ones_c, 1.0)
            warm = const.tile([1, 1], FP32)
            nc.scalar.activation(warm, ones_c[:1, :], Exp)
            ones_r = const.tile([1, HD], FP32R)
            nc.vector.tensor_copy(ones_r, ones_c[:1, :].to_broadcast([1, HD]))

            # ---- persistent SBUF ----
            woT_s = persist.tile([128, D], FP32R)
            qT = persist.tile([128, TOK], FP32R, tag="qT")
            kTn = persist.tile([128, TOK], FP32, tag="kTn")
            vTn = persist.tile([128, TOK], FP32, tag="vTn")
            oT = persist.tile([128, TOK], FP32R, tag="oT")

            def setup_batch(b, kT=None, va=None):
                bsl = bass.ts(b, T)
                if kT is None:
                    kT = kv.tile([128, L], FP32R, tag="kT", name=f"kT{b}")
                    nc.sync.dma_start(kT[:, :PAST], kTp[b, :, :])
                nc.vector.tensor_copy(kT[:, PAST:], kTn[:, bsl])
                if va is None:
                    va = kv.tile(
                        [128, HPC, NCH, HD + 1], FP32R, tag="va", name=f"va{b}"
                    )
                    nc.sync.dma_start(va[:, :, :PCH, :], vp[b, :, :, :, :])
                nc.vector.tensor_copy(
                    va[:, :, PCH:, HD],
                    ones_c[:, :, None].to_broadcast([128, HPC, NCH - PCH]),
                )
                for h in range(HPC):
                    hsl = slice(h * HD, (h + 1) * HD)
                    for tt in range(T // 128):
                        tp = flexp.tile([128, 512], FP32, tag="flex")
                        nc.tensor.transpose(
                            tp[:, :HD],
                            vTn[hsl, b * T + tt * 128 : b * T + (tt + 1) * 128],
                            identity[hsl, hsl],
                        )
                        nc.vector.tensor_copy(va[:, h, PCH + tt, :HD], tp[:, :HD])
                return kT, va

            # ---- phase A: projections (q/k/v for this core's 2 heads) ----
            nxt = None
            with (
                tc.tile_pool(name="xw", bufs=1) as xw,
                tc.tile_pool(name="xs", bufs=2) as xs,
            ):
                xT_r = xT.rearrange("(ko p) t -> p ko t", p=128)
                w_s = {}
                for name, w in (("q", wqT), ("k", wkT), ("v", wvT)):
                    w_s[name] = xw.tile(
                        [128, D // 128, 128], FP32R, tag=f"w{name}", name=f"w{name}"
                    )
                    if name == "q":
                        nc.sync.dma_start(
                            w_s[name], w.rearrange("(ko p) m -> p ko m", p=128)
                        )
                        xT_s0 = xs.tile([128, D // 128, 512], FP32R, tag="xT")
                        half = D // 256
                        nc.sync.dma_start(
                            xT_s0[:, :half, :], xT_r[:, :half, :512]
                        )
                        nc.sync.dma_start(
                            xT_s0[:, half:, :], xT_r[:, half:, :512]
                        )
                kT0 = kv.tile([128, L], FP32R, tag="kT", name="kT0")
                nc.sync.dma_start(kT0[:, : 12 * 128], kTp[0, :, : 12 * 128])
                va0 = kv.tile(
                    [128, HPC, NCH, HD + 1], FP32R, tag="va", name="va0"
                )
                nc.sync.dma_start(va0[:, :, :12, :], vp[0, :, :, :12, :])
                for name, w in (("k", wkT), ("v", wvT)):
                    nc.sync.dma_start(
                        w_s[name], w.rearrange("(ko p) m -> p ko m", p=128)
                    )
                nc.sync.dma_start(kT0[:, 12 * 128 : PAST], kTp[0, :, 12 * 128 :])
                nc.sync.dma_start(va0[:, :, 12:PCH, :], vp[0, :, :, 12:, :])
                for tcn in range(TOK // 512):
                    if tcn == 0:
                        xT_s = xT_s0
                    else:
                        xT_s = xs.tile([128, D // 128, 512], FP32R, tag="xT")
                        half = D // 256
                        nc.sync.dma_start(
                            xT_s[:, :half, :], xT_r[:, :half, bass.ts(tcn, 512)]
                        )
                        nc.sync.dma_start(
                            xT_s[:, half:, :], xT_r[:, half:, bass.ts(tcn, 512)]
                        )
                    for name, dst in (("q", qT), ("k", kTn), ("v", vTn)):
                        ps = flexp.tile([128, 512], FP32, tag="flex")
                        for ko in range(D // 128):
                            nc.tensor.matmul(
                                ps,
                                lhsT=w_s[name][:, ko, :],
                                rhs=xT_s[:, ko, :],
                                start=(ko == 0),
                                stop=(ko == D // 128 - 1),
                            )
                        nc.vector.tensor_copy(dst[:, bass.ts(tcn, 512)], ps)
                    if tcn == 0:
                        nxt = setup_batch(0, kT=kT0, va=va0)

            nc.sync.dma_start(woT_s, woT[:, :])

            # ---- phase B: attention per (batch, head) ----
            with tc.tile_pool(name="sc_ps", bufs=2, space="PSUM") as scp:
                for b in range(B):
                    bsl = bass.ts(b, T)
                    kT, va = nxt
                    if b + 1 < B:
                        nxt = setup_batch(b + 1)

                    for h in range(HPC):
                        hsl = slice(h * HD, (h + 1) * HD)
                        acc = accp.tile([HD + 1, 512], FP32, tag="acc")
                        qTh = qT[hsl, bsl]
                        for g in groups:
                            ng = len(g)
                            ps = scp.tile([128, 3 * 512], FP32, tag="sc")
                            for j, cc in enumerate(g):
                                masked = cc >= PCH
                                # queries < off see nothing from chunk cc
                                off = max(0, (cc - PCH) * 128)
                                nc.tensor.matmul(
                                    ps[:, j * 512 + off : (j + 1) * 512],
                                    lhsT=kT[hsl, bass.ts(cc, 128)],
                                    rhs=qTh[:, off:],
                                    start=True,
                                    stop=not masked,
                                )
                                if masked:
                                    nc.tensor.matmul(
                                        ps[:, j * 512 + off : (j + 1) * 512],
                                        lhsT=identity_r,
                                        rhs=masks[cc - PCH][:, off:],
                                        start=False,
                                        stop=True,
                                        skip_group_check=True,
                                    )
                            pT = ptp.tile([128, 3 * 512], FP32R, tag="pT")
                            nc.scalar.activation(
                                pT[:, : ng * 512], ps[:, : ng * 512], Exp
                            )
                            for j, cc in enumerate(g):
                                off = max(0, (cc - PCH) * 128)
                                nc.tensor.matmul(
                                    acc[:, off:],
                                    lhsT=va[:, h, cc, :],
                                    rhs=pT[:, j * 512 + off : (j + 1) * 512],
                                    start=(cc == 0),
                                    stop=(cc == NCH - 1),
                                    skip_group_check=True,
                                )
                        # evict accumulator to SBUF at once (frees the PSUM
                        # bank for the next head); denominator in row 64
                        asb = divp.tile([HD + 1, 512], FP32, tag="asb")
                        nc.vector.tensor_copy(asb, acc)
                        if b == B - 1 and h == HPC - 1:
                            r0r = divp.tile([1, 512], FP32R, tag="r0r")
                            with nc.allow_low_precision(
                                reason="fp32r reciprocal feeds broadcast matmul"
                            ):
                                nc.vector.reciprocal(r0r, asb[HD : HD + 1, :])
                            bcp = flexp.tile(
                                [HD, 512], FP32, tag="flex", name="bcp"
                            )
                            nc.tensor.matmul(
                                bcp, lhsT=ones_r, rhs=r0r, start=True, stop=True
                            )
                            nc.vector.tensor_tensor(
                                oT[hsl, bsl], asb[:HD, :], bcp, mult
                            )
                        else:
                            r0 = divp.tile([1, 512], FP32, tag="r0")
                            nc.vector.reciprocal(r0, asb[HD : HD + 1, :])
                            bc = divp.tile([HD, 512], FP32, tag="bc")
                            nc.gpsimd.partition_broadcast(bc, r0)
                            nc.vector.tensor_tensor(
                                oT[hsl, bsl], asb[:HD, :], bc, mult
                            )

                    # ---- column-parallel o_proj for this batch ----
                    out_r = out[bsl, :].rearrange("(tt p) d -> p tt d", p=128)
                    if b == B - 1:
                        ostl = stage.tile(
                            [128, T // 128, D], FP32, tag="ost", name="ostl"
                        )
                        for tt in range(T // 128):
                            tsl = slice(b * T + tt * 128, b * T + (tt + 1) * 128)
                            for nh in range(2):
                                ps = scp.tile([128, 3 * 512], FP32, tag="sc")
                                ps = ps[:, :512]
                                nc.tensor.matmul(
                                    ps,
                                    lhsT=oT[:, tsl],
                                    rhs=woT_s[:, bass.ts(nh, 512)],
                                    start=True,
                                    stop=True,
                                )
                                if nh == 1:
                                    nc.scalar.copy(
                                        ostl[:, tt, bass.ts(nh, 512)], ps
                                    )
                                else:
                                    nc.vector.tensor_copy(
                                        ostl[:, tt, bass.ts(nh, 512)], ps
                                    )
                            nc.sync.dma_start(out_r[:, tt, :], ostl[:, tt, :])
                    else:
                        ost = stage.tile([128, T // 128, D], FP32, tag="ost")
                        for tt in range(T // 128):
                            tsl = slice(b * T + tt * 128, b * T + (tt + 1) * 128)
                            for nh in range(2):
                                ps = flexp.tile([128, 512], FP32, tag="flex")
                                nc.tensor.matmul(
                                    ps,
                                    lhsT=oT[:, tsl],
                                    rhs=woT_s[:, bass.ts(nh, 512)],
                                    start=True,
                                    stop=True,
                                )
                                nc.vector.tensor_copy(
                                    ost[:, tt, bass.ts(nh, 512)], ps
                                )
                            nc.sync.dma_start(out_r[:, tt, :], ost[:, tt, :])

    nc.compile()
    return nc


def _pack_v(v):
    """[B, HPC, PAST, HD] -> [B, 128, HPC, PCH, HD+1] with ones in col HD."""
    out = np.empty((B, 128, HPC, PCH, HD + 1), np.float32)
    # v[b, h, c*128 + p, hd] -> out[b, p, h, c, hd]
    out[..., :HD] = v.reshape(B, HPC, PCH, 128, HD).transpose(0, 3, 1, 2, 4)
    out[..., HD] = 1.0
    return np.ascontiguousarray(out)


def _prep(x, k_prev, v_prev, Wq, Wk, Wv, Wo):
    """Host-side shard + layout marshalling (fp32, C-contiguous)."""
    f = np.float32
    x2 = np.ascontiguousarray(np.asarray(x, f).reshape(TOK, D))
    xT = np.ascontiguousarray(x2.T)
    k_prev = np.asarray(k_prev, f)
    v_prev = np.asarray(v_prev, f)
    Wq, Wk, Wv, Wo = (np.asarray(w, f) for w in (Wq, Wk, Wv, Wo))
    in_maps = []
    for c in range(NCORES):
        rows = slice(128 * c, 128 * (c + 1))
        hsl = slice(HPC * c, HPC * (c + 1))
        in_maps.append(
            {
                "xT": xT,
                "wqT": np.ascontiguousarray((Wq[rows, :] * SCALE).T),
                "wkT": np.ascontiguousarray(Wk[rows, :].T),
                "wvT": np.ascontiguousarray(Wv[rows, :].T),
                "woT": np.ascontiguousarray(Wo[:, rows].T),
                "kTp": np.ascontiguousarray(
                    k_prev[:, hsl, :, :].transpose(0, 1, 3, 2)
                ).reshape(B, 128, PAST),
                "vp": _pack_v(v_prev[:, hsl, :, :]),
            }
        )
    return in_maps


def kernel(x, k_prev, v_prev, Wq, Wk, Wv, Wo):
    if "nc" not in _cache:
        _cache["nc"] = _build()
    nc = _cache["nc"]
    in_maps = _prep(x, k_prev, v_prev, Wq, Wk, Wv, Wo)
    res = run_bass_kernel_spmd(nc, in_maps, core_ids=list(range(NCORES)))
    acc = np.zeros((TOK, D), np.float64)
    for r in res.results:
        acc += r["out"]
    return acc.astype(np.float32).reshape(B, T, D)


# revision 25
# speedup vs baseline: 1.0067x; 1.0040x over previous
"""Multi-head attention with KV cache, sharded over 8 NeuronCores by head.

Problem (hardcoded shapes):
  x       [4, 512, 1024]      hidden states (B, T, D)
  k_prev  [4, 16, 3584, 64]   KV cache (B, H, PAST, HD)
  v_prev  [4, 16, 3584, 64]
  Wq/Wk/Wv/Wo [1024, 1024]    projection weights (torch Linear: y = x @ W.T)

Sharding: 16 heads / 8 cores = 2 heads per core (data stays full along batch).
Each core computes q/k/v projections for its 2 heads (column-parallel),
full attention for its heads, and a column-parallel o_proj partial
[2048, 1024]; the host sums the 8 partials (the o_proj all-reduce).

Device algorithm per core (all fp32):
  - qT/kT_new/vT_new = W_slice @ x^T   via PE, contracting D (xT fed from host)
  - per (batch, head): scores^T[key, q] = k @ q^T (K=HD=64 on partitions),
    streamed in key-chunks of 128; causal mask added on the 4 newest chunks;
    exp on ScalarE (no max subtraction -- scores are O(1) by construction);
    out^T[hd, q] accumulated in PSUM via lhsT = [v | 1] so row 64 of the
    accumulator is the softmax denominator; divide, then o_proj.
"""

import numpy as np

import concourse.bass as bass
import concourse.mybir as mybir
import concourse.tile as tile
from concourse import bacc
from concourse.bass_utils import run_bass_kernel_spmd
from concourse.masks import make_identity

B, T, D = 4, 512, 1024
H, HD = 16, 64
PAST = 3584
L = PAST + T            # 4096 == MAX_CACHE, nothing is trimmed
SCALE = 1.0 / np.sqrt(HD).astype(np.float32)
NCORES = 8
HPC = H // NCORES       # heads per core = 2
TOK = B * T             # 2048
NCH = L // 128          # 32 key chunks per (b, h)
PCH = PAST // 128       # 28 chunks from the cache
FP32 = mybir.dt.float32
NEG = -1.0e30

_cache = {}

# float32r: 4-byte fp32 variant the PE consumes at full rate (~12-13 mantissa
# bits effective, measured) vs 4 cycles/row for fp32. All matmul operands are
# stored as fp32r; producers (DMA / DVE copy / ACT exp) write the rounded form.
FP32R = mybir.dt.float32r


def _build():
    nc = bacc.Bacc(None, target_bir_lowering=False)

    xT = nc.dram_tensor("xT", [D, TOK], FP32R, kind="ExternalInput")
    wqT = nc.dram_tensor("wqT", [D, 128], FP32R, kind="ExternalInput")
    wkT = nc.dram_tensor("wkT", [D, 128], FP32R, kind="ExternalInput")
    wvT = nc.dram_tensor("wvT", [D, 128], FP32R, kind="ExternalInput")
    woT = nc.dram_tensor("woT", [128, D], FP32R, kind="ExternalInput")
    kTp = nc.dram_tensor("kTp", [B, 128, PAST], FP32R, kind="ExternalInput")
    vp = nc.dram_tensor("vp", [B, 128, HPC, PCH, HD + 1], FP32R, kind="ExternalInput")
    out = nc.dram_tensor("out", [TOK, D], FP32, kind="ExternalOutput")

    Exp = mybir.ActivationFunctionType.Exp
    mult = mybir.AluOpType.mult
    add = mybir.AluOpType.add

    # key-chunk groups: scores psum tiles hold up to 3 chunks (3 PSUM banks)
    groups = [list(range(s, min(s + 3, NCH))) for s in range(0, NCH, 3)]

    with tile.TileContext(nc) as tc:
        with (
            tc.tile_pool(name="const", bufs=1) as const,
            tc.tile_pool(name="persist", bufs=1) as persist,
            tc.tile_pool(name="kv", bufs=2) as kv,
            tc.tile_pool(name="pt", bufs=2) as ptp,
            tc.tile_pool(name="div", bufs=2) as divp,
            tc.tile_pool(name="stage", bufs=1) as stage,
            tc.tile_pool(name="acc_ps", bufs=1, space="PSUM") as accp,
            tc.tile_pool(name="flex_ps", bufs=1, space="PSUM") as flexp,
        ):
            # ---- constants ----
            identity = const.tile([128, 128], FP32)
            make_identity(nc, identity)
            identity_r = const.tile([128, 128], FP32R)
            nc.vector.tensor_copy(identity_r, identity)
            masks = []
            for r in range(4):
                m = const.tile([128, T], FP32, tag=f"mask{r}")
                nc.gpsimd.memset(m, 0.0)
                # keep 0 where query i >= key-token (128r + kk), else NEG
                nc.gpsimd.affine_select(
                    out=m, in_=m, compare_op=mybir.AluOpType.is_ge,
                    fill=NEG, base=-128 * r, channel_multiplier=-1,
                    pattern=[[1, T]],
                )
                mr = const.tile([128, T], FP32R, tag=f"maskr{r}", name=f"maskr{r}")
                nc.vector.tensor_copy(mr, m)
                masks.append(mr)

            ones_c = const.tile([128, 1], FP32)
            nc.gpsimd.memset(ones_c, 1.0)
            warm = const.tile([1, 1], FP32)
            nc.scalar.activation(warm, ones_c[:1, :], Exp)
            ones_r = const.tile([1, HD], FP32R)
            nc.vector.tensor_copy(ones_r, ones_c[:1, :].to_broadcast([1, HD]))

            # ---- persistent SBUF ----
            woT_s = persist.tile([128, D], FP32R)
            qT = persist.tile([128, TOK], FP32R, tag="qT")
            kTn = persist.tile([128, TOK], FP32, tag="kTn")
            vTn = persist.tile([128, TOK], FP32, tag="vTn")
            oT = persist.tile([128, TOK], FP32R, tag="oT")

            def setup_batch(b, kT=None, va=None):
                bsl = bass.ts(b, T)
                if kT is None:
                    kT = kv.tile([128, L], FP32R, tag="kT", name=f"kT{b}")
                    nc.sync.dma_start(kT[:, :PAST], kTp[b, :, :])
                nc.vector.tensor_copy(kT[:, PAST:], kTn[:, bsl])
                if va is None:
                    va = kv.tile(
                        [128, HPC, NCH, HD + 1], FP32R, tag="va", name=f"va{b}"
                    )
                    nc.sync.dma_start(va[:, :, :PCH, :], vp[b, :, :, :, :])
                nc.vector.tensor_copy(
                    va[:, :, PCH:, HD],
                    ones_c[:, :, None].to_broadcast([128, HPC, NCH - PCH]),
                )
                for h in range(HPC):
                    hsl = slice(h * HD, (h + 1) * HD)
                    for tt in range(T // 128):
                        tp = flexp.tile([128, 512], FP32, tag="flex")
                        nc.tensor.transpose(
                            tp[:, :HD],
                            vTn[hsl, b * T + tt * 128 : b * T + (tt + 1) * 128],
                            identity[hsl, hsl],
                        )
                        nc.vector.tensor_copy(va[:, h, PCH + tt, :HD], tp[:, :HD])
                return kT, va

            # ---- phase A: projections (q/k/v for this core's 2 heads) ----
            nxt = None
            with (
                tc.tile_pool(name="xw", bufs=1) as xw,
                tc.tile_pool(name="xs", bufs=1) as xs,
            ):
                xT_r = xT.rearrange("(ko p) t -> p ko t", p=128)
                w_s = {}
                for name, w in (("q", wqT), ("k", wkT), ("v", wvT)):
                    w_s[name] = xw.tile(
                        [128, D // 128, 128], FP32R, tag=f"w{name}", name=f"w{name}"
                    )
                    if name == "q":
                        nc.sync.dma_start(
                            w_s[name], w.rearrange("(ko p) m -> p ko m", p=128)
                        )
                        xT_s0 = xs.tile([128, D // 128, 512], FP32R, tag="xT")
                        half = D // 256
                        nc.sync.dma_start(
                            xT_s0[:, :half, :], xT_r[:, :half, :512]
                        )
                        nc.sync.dma_start(
                            xT_s0[:, half:, :], xT_r[:, half:, :512]
                        )
                kT0 = kv.tile([128, L], FP32R, tag="kT", name="kT0")
                nc.sync.dma_start(kT0[:, : 12 * 128], kTp[0, :, : 12 * 128])
                va0 = kv.tile(
                    [128, HPC, NCH, HD + 1], FP32R, tag="va", name="va0"
                )
                nc.sync.dma_start(va0[:, :, :12, :], vp[0, :, :, :12, :])
                for name, w in (("k", wkT), ("v", wvT)):
                    nc.sync.dma_start(
                        w_s[name], w.rearrange("(ko p) m -> p ko m", p=128)
                    )
                nc.sync.dma_start(kT0[:, 12 * 128 : PAST], kTp[0, :, 12 * 128 :])
                nc.sync.dma_start(va0[:, :, 12:PCH, :], vp[0, :, :, 12:, :])
                def proj_tc(tcn, xT_s=None):
                    if xT_s is None:
                        xT_s = xs.tile(
                            [128, D // 128, 512], FP32R, tag="xT", name="xT_s"
                        )
                        half = D // 256
                        nc.sync.dma_start(
                            xT_s[:, :half, :], xT_r[:, :half, bass.ts(tcn, 512)]
                        )
                        nc.sync.dma_start(
                            xT_s[:, half:, :], xT_r[:, half:, bass.ts(tcn, 512)]
                        )
                    for name, dst in (("q", qT), ("k", kTn), ("v", vTn)):
                        ps = flexp.tile([128, 512], FP32, tag="flex")
                        for ko in range(D // 128):
                            nc.tensor.matmul(
                                ps,
                                lhsT=w_s[name][:, ko, :],
                                rhs=xT_s[:, ko, :],
                                start=(ko == 0),
                                stop=(ko == D // 128 - 1),
                            )
                        nc.vector.tensor_copy(dst[:, bass.ts(tcn, 512)], ps)

                proj_tc(0, xT_s=xT_s0)
                nxt = setup_batch(0, kT=kT0, va=va0)
                proj_tc(1)

                nc.sync.dma_start(woT_s, woT[:, :])

                # ---- phase B: attention per (batch, head) ----
                scp_cm = tc.tile_pool(name="sc_ps", bufs=2, space="PSUM")
                scp = scp_cm.__enter__()
                for b in range(B):
                    bsl = bass.ts(b, T)
                    kT, va = nxt
                    if b + 2 < B:
                        proj_tc(b + 2)
                    if b + 1 < B:
                        nxt = setup_batch(b + 1)

                    for h in range(HPC):
                        hsl = slice(h * HD, (h + 1) * HD)
                        acc = accp.tile([HD + 1, 512], FP32, tag="acc")
                        qTh = qT[hsl, bsl]
                        for g in groups:
                            ng = len(g)
                            ps = scp.tile([128, 3 * 512], FP32, tag="sc")
                            for j, cc in enumerate(g):
                                masked = cc >= PCH
                                # queries < off see nothing from chunk cc
                                off = max(0, (cc - PCH) * 128)
                                nc.tensor.matmul(
                                    ps[:, j * 512 + off : (j + 1) * 512],
                                    lhsT=kT[hsl, bass.ts(cc, 128)],
                                    rhs=qTh[:, off:],
                                    start=True,
                                    stop=not masked,
                                )
                                if masked:
                                    nc.tensor.matmul(
                                        ps[:, j * 512 + off : (j + 1) * 512],
                                        lhsT=identity_r,
                                        rhs=masks[cc - PCH][:, off:],
                                        start=False,
                                        stop=True,
                                        skip_group_check=True,
                                    )
                            pT = ptp.tile([128, 3 * 512], FP32R, tag="pT")
                            nc.scalar.activation(
                                pT[:, : ng * 512], ps[:, : ng * 512], Exp
                            )
                            for j, cc in enumerate(g):
                                off = max(0, (cc - PCH) * 128)
                                nc.tensor.matmul(
                                    acc[:, off:],
                                    lhsT=va[:, h, cc, :],
                                    rhs=pT[:, j * 512 + off : (j + 1) * 512],
                                    start=(cc == 0),
                                    stop=(cc == NCH - 1),
                                    skip_group_check=True,
                                )
                        # evict accumulator to SBUF at once (frees the PSUM
                        # bank for the next head); denominator in row 64
                        asb = divp.tile([HD + 1, 512], FP32, tag="asb")
                        nc.vector.tensor_copy(asb, acc)
                        if b == B - 1 and h == HPC - 1:
                            r0r = divp.tile([1, 512], FP32R, tag="r0r")
                            with nc.allow_low_precision(
                                reason="fp32r reciprocal feeds broadcast matmul"
                            ):
                                nc.vector.reciprocal(r0r, asb[HD : HD + 1, :])
                            bcp = flexp.tile(
                                [HD, 512], FP32, tag="flex", name="bcp"
                            )
                            nc.tensor.matmul(
                                bcp, lhsT=ones_r, rhs=r0r, start=True, stop=True
                            )
                            nc.vector.tensor_tensor(
                                oT[hsl, bsl], asb[:HD, :], bcp, mult
                            )
                        else:
                            r0 = divp.tile([1, 512], FP32, tag="r0")
                            nc.vector.reciprocal(r0, asb[HD : HD + 1, :])
                            bc = divp.tile([HD, 512], FP32, tag="bc")
                            nc.gpsimd.partition_broadcast(bc, r0)
                            nc.vector.tensor_tensor(
                                oT[hsl, bsl], asb[:HD, :], bc, mult
                            )

                    # ---- column-parallel o_proj for this batch ----
                    out_r = out[bsl, :].rearrange("(tt p) d -> p tt d", p=128)
                    if b == B - 1:
                        ostl = stage.tile(
                            [128, T // 128, D], FP32, tag="ost", name="ostl"
                        )
                        for tt in range(T // 128):
                            tsl = slice(b * T + tt * 128, b * T + (tt + 1) * 128)
                            for nh in range(2):
                                ps = scp.tile([128, 3 * 512], FP32, tag="sc")
                                ps = ps[:, :512]
                                nc.tensor.matmul(
                                    ps,
                                    lhsT=oT[:, tsl],
                                    rhs=woT_s[:, bass.ts(nh, 512)],
                                    start=True,
                                    stop=True,
                                )
                                if nh == 1:
                                    nc.scalar.copy(
                                        ostl[:, tt, bass.ts(nh, 512)], ps
                                    )
                                else:
                                    nc.vector.tensor_copy(
                                        ostl[:, tt, bass.ts(nh, 512)], ps
                                    )
                            nc.sync.dma_start(out_r[:, tt, :], ostl[:, tt, :])
                    else:
                        ost = stage.tile([128, T // 128, D], FP32, tag="ost")
                        for tt in range(T // 128):
                            tsl = slice(b * T + tt * 128, b * T + (tt + 1) * 128)
                            for nh in range(2):
                                ps = flexp.tile([128, 512], FP32, tag="flex")
                                nc.tensor.matmul(
                                    ps,
                                    lhsT=oT[:, tsl],
                                    rhs=woT_s[:, bass.ts(nh, 512)],
                                    start=True,
                                    stop=True,
                                )
                                nc.vector.tensor_copy(
                                    ost[:, tt, bass.ts(nh, 512)], ps
                                )
                            nc.sync.dma_start(out_r[:, tt, :], ost[:, tt, :])
                scp_cm.__exit__(None, None, None)

    nc.compile()
    return nc


def _pack_v(v):
    """[B, HPC, PAST, HD] -> [B, 128, HPC, PCH, HD+1] with ones in col HD."""
    out = np.empty((B, 128, HPC, PCH, HD + 1), np.float32)
    # v[b, h, c*128 + p, hd] -> out[b, p, h, c, hd]
    out[..., :HD] = v.reshape(B, HPC, PCH, 128, HD).transpose(0, 3, 1, 2, 4)
    out[..., HD] = 1.0
    return np.ascontiguousarray(out)


def _prep(x, k_prev, v_prev, Wq, Wk, Wv, Wo):
    """Host-side shard + layout marshalling (fp32, C-contiguous)."""
    f = np.float32
    x2 = np.ascontiguousarray(np.asarray(x, f).reshape(TOK, D))
    xT = np.ascontiguousarray(x2.T)
    k_prev = np.asarray(k_prev, f)
    v_prev = np.asarray(v_prev, f)
    Wq, Wk, Wv, Wo = (np.asarray(w, f) for w in (Wq, Wk, Wv, Wo))
    in_maps = []
    for c in range(NCORES):
        rows = slice(128 * c, 128 * (c + 1))
        hsl = slice(HPC * c, HPC * (c + 1))
        in_maps.append(
            {
                "xT": xT,
                "wqT": np.ascontiguousarray((Wq[rows, :] * SCALE).T),
                "wkT": np.ascontiguousarray(Wk[rows, :].T),
                "wvT": np.ascontiguousarray(Wv[rows, :].T),
                "woT": np.ascontiguousarray(Wo[:, rows].T),
                "kTp": np.ascontiguousarray(
                    k_prev[:, hsl, :, :].transpose(0, 1, 3, 2)
                ).reshape(B, 128, PAST),
                "vp": _pack_v(v_prev[:, hsl, :, :]),
            }
        )
    return in_maps


def kernel(x, k_prev, v_prev, Wq, Wk, Wv, Wo):
    if "nc" not in _cache:
        _cache["nc"] = _build()
    nc = _cache["nc"]
    in_maps = _prep(x, k_prev, v_prev, Wq, Wk, Wv, Wo)
    res = run_bass_kernel_spmd(nc, in_maps, core_ids=list(range(NCORES)))
    acc = np.zeros((TOK, D), np.float64)
    for r in res.results:
        acc += r["out"]
    return acc.astype(np.float32).reshape(B, T, D)
